# revision 1
# baseline (speedup 1.0000x reference)
"""LIFSpike Trainium2 kernel (Bass/Tile), SPMD over 8 NeuronCores.

Reference semantics (T=4, tau=2, vth=1, vreset=0, decay_input=False,
hard reset):
    xs = x.reshape(T, B//T, C, H, W)
    v0 = 0
    h_t = v_t * 0.5 + x_t
    s_t = (h_t >= 1.0)
    v_{t+1} = h_t * (h_t < 1.0)
    out = s.reshape(B, C, H, W)

Kernel-side reformulation (exact in fp32 -- all rescalings are by powers
of two, which commute with fp rounding):
    r_t := 2^t * h_t,  host supplies x'_t = 2^t * x_t
    r_0     = x'_0                                    (plain DMA load)
    s_t     = (r_t >= 2^t)                            (tensor_scalar, ge)
    q_t     = (r_t < 2^t) * r_t   (= 2^t * v_{t+1})   (one STT op)
    r_{t+1} = q_t + x'_{t+1}      (= 2^{t+1} * h_{t+1}) (DMA accum add)

Sharding: data-parallel over the per-timestep batch dim B//T = 32 ->
4 chains per core. Per core each timestep is one [128, 8192] f32 tile
(4 chains x 2048 free elements); the scan over T is local.

Host-side layout per core (partition-major, t-major):
    x_core[p, t*8192 + b*2048 + j] = 0.5 * x[t*32 + core*4 + b, flat=p*2048+j]

Output is computed as bf16 (values are exactly 0.0/1.0) to halve store
bandwidth, and cast back to f32 on the host.
"""

import numpy as np

T = 4
BP = 32               # B // T
NCORES = 8
BPC = BP // NCORES    # chains per core = 4
SLICE = 256 * 32 * 32  # elements per (t, b) slice = 262144
P = 128
FREE_B = SLICE // P   # 2048
FREE_T = BPC * FREE_B  # 8192
FREE = T * FREE_T     # 32768

_cache = {}


def _build_program():
    import concourse.bass as bass
    import concourse.tile as tile
    from concourse import bacc, mybir

    Alu = mybir.AluOpType
    f32 = mybir.dt.float32
    bf16 = mybir.dt.bfloat16

    # Bacc (not plain Bass): its compile() pass splits multi-sem waits into
    # event-semaphore chains -- the TRN2 ISA allows only one wait per
    # instruction.
    nc = bacc.Bacc(debug=False)
    x = nc.dram_tensor("x", [P, FREE], f32, kind="ExternalInput").ap()
    s = nc.dram_tensor("s", [P, FREE], bf16, kind="ExternalOutput").ap()

    with tile.TileContext(nc) as tc:
        with (
            tc.tile_pool(name="state", bufs=1) as vpool,
            tc.tile_pool(name="sout", bufs=2) as spool,
        ):
            # ping-pong state buffers: p lives in one, q is written to the
            # other, then the next x' chunk is DMA-accumulated on top of q.
            pA = vpool.tile([P, FREE_T], f32)
            pB = vpool.tile([P, FREE_T], f32)
            bufs = [pA, pB]
            # outputs for two timesteps share one tile so each store DMA
            # covers both (fewer DMA sem lanes -> final drain fits the
            # single-sync-wait-per-instruction ISA budget).
            souts = [
                spool.tile([P, 2 * FREE_T], bf16, tag=f"s{i}", name=f"sout{i}")
                for i in range(T // 2)
            ]
            nc.sync.dma_start(pA[:], x[:, 0:FREE_T])
            # accum DMAs crash the device above ~1 MiB -> chunk them
            CH = 2048
            for t in range(T):
                th = float(1 << t)  # threshold 2^t
                p = bufs[t % 2]
                st = souts[t // 2][:, (t % 2) * FREE_T:(t % 2 + 1) * FREE_T]
                nc.vector.tensor_scalar(st, p[:], th, None, Alu.is_ge)
                if t < T - 1:
                    q = bufs[(t + 1) % 2]
                    nc.vector.scalar_tensor_tensor(
                        q[:], p[:], th, p[:], Alu.is_lt, Alu.mult
                    )
                    base = (t + 1) * FREE_T
                    for c in range(0, FREE_T, CH):
                        nc.gpsimd.dma_start(
                            q[:, c:c + CH],
                            x[:, base + c:base + c + CH],
                            accum_op=Alu.add,
                        )
                if t % 2 == 1:
                    nc.sync.dma_start(
                        s[:, (t - 1) * FREE_T:(t + 1) * FREE_T],
                        souts[t // 2][:],
                    )
    nc.compile()
    return nc


def _shard(x):
    # x: (128, 256, 32, 32) f32 -> list of 8 per-core [128, 32768] arrays,
    # timestep t pre-scaled by 2^t (exact in fp32)
    xr = np.ascontiguousarray(x).reshape(T, BP, SLICE)
    tscale = (2.0 ** np.arange(T, dtype=np.float32)).astype(np.float32)
    shards = []
    for k in range(NCORES):
        xk = xr[:, k * BPC:(k + 1) * BPC, :].reshape(T, BPC, P, FREE_B)
        xk = xk * tscale[:, None, None, None]
        xk = xk.transpose(2, 0, 1, 3).reshape(P, FREE)
        shards.append(np.asarray(xk, dtype=np.float32))
    return shards


def _unshard(parts):
    # parts: 8 per-core [128, 32768] arrays (bf16) -> (128,256,32,32) f32
    out = np.empty((T, BP, SLICE), dtype=np.float32)
    for k, sk in enumerate(parts):
        sk = np.asarray(sk).astype(np.float32).reshape(P, T, BPC, FREE_B)
        out[:, k * BPC:(k + 1) * BPC, :] = (
            sk.transpose(1, 2, 0, 3).reshape(T, BPC, SLICE)
        )
    return out.reshape(T * BP, 256, 32, 32)


def kernel(x):
    from concourse.bass_utils import run_bass_kernel_spmd

    if "nc" not in _cache:
        _cache["nc"] = _build_program()
    nc = _cache["nc"]

    shards = _shard(np.asarray(x, dtype=np.float32))
    in_maps = [{"x": sk} for sk in shards]
    res = run_bass_kernel_spmd(nc, in_maps, list(range(NCORES)))
    return _unshard([res.results[k]["s"] for k in range(NCORES)])



# revision 2
# speedup vs baseline: 1.6105x; 1.6105x over previous
"""LIFSpike Trainium2 kernel (Bass/Tile), SPMD over 8 NeuronCores.

Reference semantics (T=4, tau=2, vth=1, vreset=0, decay_input=False,
hard reset):
    xs = x.reshape(T, B//T, C, H, W)
    v0 = 0
    h_t = v_t * 0.5 + x_t
    s_t = (h_t >= 1.0)
    v_{t+1} = h_t * (h_t < 1.0)
    out = s.reshape(B, C, H, W)

Kernel-side reformulation (exact in fp32 -- all rescalings are by powers
of two, which commute with fp rounding):
    r_t := 2^t * h_t,  host supplies x'_t = 2^t * x_t
    r_0     = x'_0                                    (plain DMA load)
    s_t     = (r_t >= 2^t)                            (tensor_scalar, ge)
    q_t     = (r_t < 2^t) * r_t   (= 2^t * v_{t+1})   (one STT op)
    r_{t+1} = q_t + x'_{t+1}      (= 2^{t+1} * h_{t+1}) (DMA accum add)

Sharding: data-parallel over the per-timestep batch dim B//T = 32 ->
4 independent scan chains per core, each a [128, 2048] f32 tile per
timestep.  The 4 chains are software-pipelined: while chain b runs its
DVE ops for timestep t, the accum DMAs / loads of the other chains are
in flight, keeping the DMA engines (the roofline resource: 16 MiB in +
4 MiB out per core) saturated.

Host-side input layout per core (partition-major, t-major):
    x_core[p, t*8192 + b*2048 + j] = 2^t * x[t*32 + core*4 + b, flat=p*2048+j]
Output layout is b-major so each chain stores once, contiguously:
    s_core[p, b*8192 + t*2048 + j]
Output is computed as fp8e4 (values are exactly 0.0/1.0) to quarter the
store bandwidth vs f32, and cast back to f32 on the host.
"""

import numpy as np

T = 4
BP = 32               # B // T
NCORES = 8
BPC = BP // NCORES    # chains per core = 4
SLICE = 256 * 32 * 32  # elements per (t, b) slice = 262144
P = 128
W = SLICE // P        # free elems per chain-timestep tile = 2048
FREE_T = BPC * W      # 8192 (one timestep slab, all chains)
FREE = T * FREE_T     # 32768

_cache = {}


def _build_program():
    import concourse.bass as bass
    import concourse.tile as tile
    from concourse import bacc, mybir

    Alu = mybir.AluOpType
    f32 = mybir.dt.float32
    out_dt = mybir.dt.float8e4

    nc = bacc.Bacc(debug=False)
    x = nc.dram_tensor("x", [P, FREE], f32, kind="ExternalInput").ap()
    s = nc.dram_tensor("s", [P, FREE], out_dt, kind="ExternalOutput").ap()

    with tile.TileContext(nc) as tc:
        with (
            tc.tile_pool(name="state", bufs=1) as vpool,
            tc.tile_pool(name="sout", bufs=1) as spool,
        ):
            # t=0 state: one slab holding all 4 chains' first timestep
            x0 = vpool.tile([P, FREE_T], f32, tag="x0", name="x0")
            # per-(chain, t>=1) state tiles -- accum DMA lands x' on top
            qt = [
                [
                    vpool.tile([P, W], f32, tag=f"q{b}_{t}", name=f"q{b}_{t}")
                    for t in range(1, T)
                ]
                for b in range(BPC)
            ]
            # per-chain output tile: 4 timesteps side by side, one store
            outs = [
                spool.tile([P, T * W], out_dt, tag=f"s{b}", name=f"sout{b}")
                for b in range(BPC)
            ]

            # chain-ordered t0 loads so chain 0's compute starts early
            for b in range(BPC):
                nc.sync.dma_start(
                    x0[:, b * W:(b + 1) * W], x[:, b * W:(b + 1) * W]
                )

            def ptile(b, t):
                if t == 0:
                    return x0[:, b * W:(b + 1) * W]
                return qt[b][t - 1][:]

            for t in range(T):
                th = float(1 << t)  # threshold 2^t
                for b in range(BPC):
                    p = ptile(b, t)
                    if t < T - 1:
                        q = qt[b][t]
                        # v-update first: it unblocks the next accum DMA
                        nc.vector.scalar_tensor_tensor(
                            q[:], p, th, p, Alu.is_lt, Alu.mult
                        )
                        base = (t + 1) * FREE_T + b * W
                        nc.gpsimd.dma_start(
                            q[:], x[:, base:base + W], accum_op=Alu.add
                        )
                    nc.vector.tensor_scalar(
                        outs[b][:, t * W:(t + 1) * W], p, th, None, Alu.is_ge
                    )
                    if t == T - 1:
                        nc.sync.dma_start(
                            s[:, b * T * W:(b + 1) * T * W], outs[b][:]
                        )
    nc.compile()
    return nc


def _shard(x):
    # x: (128, 256, 32, 32) f32 -> list of 8 per-core [128, 32768] arrays,
    # timestep t pre-scaled by 2^t (exact in fp32)
    xr = np.ascontiguousarray(x).reshape(T, BP, SLICE)
    tscale = (2.0 ** np.arange(T, dtype=np.float32)).astype(np.float32)
    shards = []
    for k in range(NCORES):
        xk = xr[:, k * BPC:(k + 1) * BPC, :].reshape(T, BPC, P, W)
        xk = xk * tscale[:, None, None, None]
        xk = xk.transpose(2, 0, 1, 3).reshape(P, FREE)
        shards.append(np.asarray(xk, dtype=np.float32))
    return shards


def _unshard(parts):
    # parts: 8 per-core [128, 32768] arrays (fp8, b-major) -> (128,256,32,32) f32
    out = np.empty((T, BP, SLICE), dtype=np.float32)
    for k, sk in enumerate(parts):
        sk = np.asarray(sk).astype(np.float32).reshape(P, BPC, T, W)
        out[:, k * BPC:(k + 1) * BPC, :] = (
            sk.transpose(2, 1, 0, 3).reshape(T, BPC, SLICE)
        )
    return out.reshape(T * BP, 256, 32, 32)


def kernel(x):
    from concourse.bass_utils import run_bass_kernel_spmd

    if "nc" not in _cache:
        _cache["nc"] = _build_program()
    nc = _cache["nc"]

    shards = _shard(np.asarray(x, dtype=np.float32))
    in_maps = [{"x": sk} for sk in shards]
    res = run_bass_kernel_spmd(nc, in_maps, list(range(NCORES)))
    return _unshard([res.results[k]["s"] for k in range(NCORES)])
